# revision 54
# baseline (speedup 1.0000x reference)
"""MoE routing layer (8 experts, top-2, B=8 seqs) on 8 TRN2 NeuronCores.

Strategy: data-parallel over B. Core b processes sequence b with its two
routed experts. Routing (softmax over an 8x8 logit matrix) + aux_loss are
computed on host; the two selected experts' weights are gathered, transposed
to feature-major lhsT layout, cast to bf16, and shipped per-core. The device
kernel keeps activations feature-major ([feature, token]) so every matmul's
contraction dim lands on SBUF partitions; attention uses transposed scores
(k_fm.T @ q_fm), exp on ScalarE, and a ones-row-augmented V matmul that
produces numerator + softmax denominator in one PSUM tile. RoPE's
rotate_half is one extra TensorE matmul with a constant +-1 permutation
matrix. RMS-norm partition reductions and per-token broadcasts are
ones-vector matmuls on the TensorEngine.
"""

import os
import sys

import numpy as np

for _p in ("/opt/trn_rl_repo",):
    if _p not in sys.path and os.path.isdir(_p):
        sys.path.insert(0, _p)

import json

import ml_dtypes
from contextlib import ExitStack

import concourse.bass as bass
import concourse.mybir as mybir
import concourse.tile as tile
from concourse.bass_utils import run_bass_kernel_spmd

BF16 = mybir.dt.bfloat16
F32 = mybir.dt.float32
AF = mybir.ActivationFunctionType
BF = ml_dtypes.bfloat16

B, S, H = 8, 512, 1024
E, TOPK = 8, 2
NH, HD = 8, 64
HSUB = 512
INTER = 1536
EPS = 1e-5
P = 128
NCORES = 8

_CACHE = {}


def _legalize_bir_json(raw: bytes) -> bytes:
    """Hoist inline sync waits onto standalone EventSemaphore instructions.

    The walrus build in this container rejects instructions whose encoding
    has no room for inline sync-wait commands (DVE TensorTensor, Drain, ...)
    with "Too many sync wait commands". Standalone EventSemaphore waits on
    the same engine are always legal, so move every wait there.
    """
    d = json.loads(raw)
    n = 0
    for fn in d["functions"]:
        for blk in fn["blocks"]:
            out = []
            for inst in blk["instructions"]:
                si = inst.get("sync_info")
                waits = si.get("on_wait") if si else None
                if waits and not (
                    inst.get("opcode") == "EventSemaphore" and len(waits) <= 1
                ):
                    for w in waits:
                        n += 1
                        out.append(
                            {
                                "debug": inst.get("debug"),
                                "engine": inst["engine"],
                                "ins": [],
                                "outs": [],
                                "name": f"I-hw{n}",
                                "opcode": "EventSemaphore",
                                "sync_info": {"on_update": [], "on_wait": [w]},
                            }
                        )
                    si["on_wait"] = []
                out.append(inst)
            blk["instructions"] = out
    return json.dumps(d).encode()


# ---------------------------------------------------------------- device ---
def _build_nc(reps=1):
    nc = bass.Bass()

    hTbf_e = nc.declare_dram_parameter("hTbf", [P, 8, S], BF16, isOutput=False)
    hT32_e = nc.declare_dram_parameter("hT32", [P, 8, S], F32, isOutput=False)
    cos2_e = nc.declare_dram_parameter("cos2", [P, S], BF16, isOutput=False)
    sin2_e = nc.declare_dram_parameter("sin2", [P, S], BF16, isOutput=False)
    rotT_e = nc.declare_dram_parameter("rotT", [P, P], BF16, isOutput=False)
    dwT_e = nc.declare_dram_parameter("dwT", [2, P, 8, HSUB], BF16, isOutput=False)
    qkvT_e = nc.declare_dram_parameter("qkvT", [2, P, 4, 1536], BF16, isOutput=False)
    owT_e = nc.declare_dram_parameter("owT", [2, P, 4, HSUB], BF16, isOutput=False)
    guwT_e = nc.declare_dram_parameter(
        "guwT", [2, P, 4, 2 * INTER], BF16, isOutput=False
    )
    mdwT_e = nc.declare_dram_parameter("mdwT", [2, P, 12, HSUB], BF16, isOutput=False)
    uwT_e = nc.declare_dram_parameter("uwT", [2, P, 4, H], BF16, isOutput=False)
    out_e = nc.declare_dram_parameter("out", [P, 8, S], F32, isOutput=True)

    with tile.TileContext(nc) as tc, ExitStack() as ctx:
        cpool = ctx.enter_context(tc.tile_pool(name="c", bufs=1))
        wpool = ctx.enter_context(tc.tile_pool(name="w", bufs=1))
        apool = ctx.enter_context(tc.tile_pool(name="a", bufs=1))
        spool = ctx.enter_context(tc.tile_pool(name="s", bufs=2))
        otpool = ctx.enter_context(tc.tile_pool(name="ot", bufs=4))
        pmm = ctx.enter_context(tc.tile_pool(name="pmm", bufs=6, space="PSUM"))
        pred = ctx.enter_context(tc.tile_pool(name="pred", bufs=1, space="PSUM"))
        pbc = ctx.enter_context(tc.tile_pool(name="pbc", bufs=1, space="PSUM"))

        # ---- persistent tiles. h and small constants ride the Pool-engine
        # DMA queue so expert weights on the sync queue aren't stuck behind
        # them at kernel start.
        t_hbf = cpool.tile([P, 8, S], BF16, tag="hbf")
        t_cos = cpool.tile([P, S], BF16, tag="cos")
        t_sin = cpool.tile([P, S], BF16, tag="sin")
        t_rot = cpool.tile([P, P], BF16, tag="rot")
        ones_bf = cpool.tile([P, P], BF16, tag="ones_bf")
        nc.vector.memset(ones_bf, 1.0)
        ones_f = cpool.tile([P, P], F32, tag="ones_f")
        nc.vector.memset(ones_f, 1.0)
        eps_t = cpool.tile([1, 1], F32, tag="eps")
        nc.vector.memset(eps_t, EPS)
        acc = None

        def rms_bcast(x_tile, nch, scale, final=False):
            """x_tile [P, nch, S]: return PSUM [P, S] f32 = rstd broadcast."""
            red = pred.tile([1, S], F32, tag="red")
            for c in range(nch):
                sq = spool.tile([P, S], BF16, tag="sq")
                if x_tile.dtype == BF16:
                    # all-bf16 SBUF -> DVE 4x mode
                    nc.vector.tensor_mul(sq, x_tile[:, c, :], x_tile[:, c, :])
                else:
                    nc.scalar.activation(sq, x_tile[:, c, :], AF.Square)
                nc.tensor.matmul(
                    red, ones_bf[:, 0:1], sq, start=(c == 0), stop=(c == nch - 1)
                )
            pb = pbc.tile([P, S], F32, tag="bc")
            if final:
                std = spool.tile([1, S], F32, tag="std")
                nc.scalar.activation(
                    std, red, AF.Sqrt, bias=eps_t[0:1, 0:1], scale=float(scale)
                )
                rstd = spool.tile([1, S], F32, tag="std")
                nc.vector.reciprocal(rstd, std)
                nc.tensor.matmul(pb, ones_f[0:1, 0:P], rstd, start=True, stop=True)
            else:
                std = spool.tile([1, S], F32, tag="std")
                nc.scalar.activation(
                    std, red, AF.Sqrt, bias=eps_t[0:1, 0:1], scale=float(scale)
                )
                rstd = spool.tile([1, S], BF16, tag="std")
                with nc.allow_low_precision(reason="intermediate rms rstd bf16"):
                    nc.vector.reciprocal(rstd, std)
                nc.tensor.matmul(pb, ones_bf[0:1, 0:P], rstd, start=True, stop=True)
            return pb

        for rep in range(reps):
          # acc accumulates h + both experts (h added per-chunk at slot 0)
          if acc is None:
              acc = cpool.tile([P, 8, S], F32, tag="acc", name="acc")
          for slot in range(2):
            # ---- expert weights (streamed; tag-slot reuse pipelines the DMA)
            w_dw = wpool.tile([P, 8, HSUB], BF16, tag="dwT")
            if rep == 0 and slot == 0:
                # interleave dw/h chunks so the fi-outer down-proj can start
                # after the first pair and never outruns the DMA feed
                for fi in range(0, 8, 2):
                    nc.sync.dma_start(
                        out=w_dw[:, fi : fi + 2, :], in_=dwT_e[slot][:, fi : fi + 2, :]
                    )
                    nc.sync.dma_start(
                        out=t_hbf[:, fi : fi + 2, :], in_=hTbf_e[:, fi : fi + 2, :]
                    )
            else:
                # later experts: dw/qkv ride the Pool queue, parallel to the
                # sync queue still draining the previous expert's gu/md/up
                nc.gpsimd.dma_start(out=w_dw[:, 0:2, :], in_=dwT_e[slot][:, 0:2, :])
                nc.gpsimd.dma_start(out=w_dw[:, 2:8, :], in_=dwT_e[slot][:, 2:8, :])
            w_qkv = wpool.tile([P, 4, 1536], BF16, tag="qkvT")
            qkv_eng = nc.sync if (rep == 0 and slot == 0) else nc.gpsimd
            qkv_eng.dma_start(out=w_qkv, in_=qkvT_e[slot])
            if rep == 0 and slot == 0:
                # constants are first needed at RoPE, ~15us in — keep them
                # behind the first expert's critical dw/h chunks
                nc.gpsimd.dma_start(out=t_cos, in_=cos2_e[:, :])
                nc.gpsimd.dma_start(out=t_sin, in_=sin2_e[:, :])
                nc.gpsimd.dma_start(out=t_rot, in_=rotT_e[:, :])
            w_o = wpool.tile([P, 4, HSUB], BF16, tag="owT")
            nc.sync.dma_start(out=w_o, in_=owT_e[slot])
            w_gu = wpool.tile([P, 4, 2 * INTER], BF16, tag="guwT")
            nc.sync.dma_start(out=w_gu[:, :, 0:INTER], in_=guwT_e[slot][:, :, 0:INTER])
            nc.sync.dma_start(
                out=w_gu[:, :, INTER : 2 * INTER],
                in_=guwT_e[slot][:, :, INTER : 2 * INTER],
            )
            w_md = wpool.tile([P, 12, HSUB], BF16, tag="mdwT")
            nc.sync.dma_start(out=w_md, in_=mdwT_e[slot])
            w_up = wpool.tile([P, 4, H], BF16, tag="uwT")
            nc.sync.dma_start(out=w_up, in_=uwT_e[slot])

            # ---- down-proj: hs = h @ dw.T   (feature-major [HSUB, S])
            # contraction (fi) outermost over 4 held PSUM banks: the first
            # matmuls need only the first h/dw chunks off the DMA queue
            hs_bf = apool.tile([P, 4, S], BF16, tag="hs_bf")
            po4 = [
                pmm.tile([P, S], F32, tag="mm", name=f"dps{slot}_{i}")
                for i in range(4)
            ]
            for fi in range(8):
                for fo in range(4):
                    nc.tensor.matmul(
                        po4[fo],
                        w_dw[:, fi, fo * P : (fo + 1) * P],
                        t_hbf[:, fi, :],
                        start=(fi == 0),
                        stop=(fi == 7),
                    )
            for fo in range(4):
                nc.scalar.activation(hs_bf[:, fo, :], po4[fo], AF.Copy)

            # ---- qkv + RoPE (rotate_half = matmul with constant +-1 perm)
            qbf = apool.tile([P, 4, S], BF16, tag="qbf")
            kbf = apool.tile([P, 4, S], BF16, tag="kbf")

            def qkv_chunk(col0):
                ps = pmm.tile([P, S], F32, tag="mm")
                for fi in range(4):
                    nc.tensor.matmul(
                        ps,
                        w_qkv[:, fi, col0 : col0 + P],
                        hs_bf[:, fi, :],
                        start=(fi == 0),
                        stop=(fi == 3),
                    )
                return ps

            # all 8 q/k chunk matmuls first (PE never waits on ACT), then the
            # 8 rot matmuls whose ACT-copied inputs are ready by then
            qraw = apool.tile([P, 4, S], BF16, tag="qraw")
            kraw = apool.tile([P, 4, S], BF16, tag="kraw")
            for c in range(4):
                for raw, base in ((qraw, 0), (kraw, 512)):
                    ps_x = qkv_chunk(base + c * P)
                    nc.scalar.activation(raw[:, c, :], ps_x, AF.Copy)
            for c in range(4):
                for raw, dst in ((qraw, qbf), (kraw, kbf)):
                    ps_r = pmm.tile([P, S], F32, tag="mm")
                    nc.tensor.matmul(ps_r, t_rot, raw[:, c, :], start=True, stop=True)
                    # all-bf16 SBUF operands -> DVE 4x mode on t1 and the add
                    t1 = spool.tile([P, S], BF16, tag="t1")
                    nc.vector.tensor_mul(t1, raw[:, c, :], t_cos)
                    t2 = spool.tile([P, S], BF16, tag="t2")
                    nc.vector.tensor_mul(t2, ps_r, t_sin)
                    nc.vector.tensor_add(dst[:, c, :], t1, t2)

            # ---- v (token-major) + ones column for the softmax denominator
            vaug = apool.tile([P, 4, NH, 65], BF16, tag="vaug")
            nc.vector.memset(vaug[:, :, :, 64:65], 1.0)
            for tcn in range(4):
                ps = pmm.tile([P, S], F32, tag="mm")
                for fi in range(4):
                    nc.tensor.matmul(
                        ps,
                        hs_bf[:, fi, tcn * P : (tcn + 1) * P],
                        w_qkv[:, fi, 1024:1536],
                        start=(fi == 0),
                        stop=(fi == 3),
                    )
                nc.scalar.activation(
                    vaug[:, tcn, :, 0:64],
                    ps.rearrange("p (h d) -> p h d", h=NH),
                    AF.Copy,
                )

            # ---- attention, head-pair interleaved: the even/odd heads' 64-
            # contraction score matmuls sit in PE row groups 0/64 and run
            # concurrently in the array. Pairs pack into attnout2 [128, 4, S]
            # so o-proj contracts over the full 128 partitions.
            attnout2 = apool.tile([P, 4, S], BF16, tag="attnout2")
            for c in range(4):
                h0, h1 = 2 * c, 2 * c + 1
                e0 = spool.tile([P, 4, S], BF16, tag="expT")
                e1 = spool.tile([P, 4, S], BF16, tag="expT2")
                for tcn in range(4):
                    ps0 = pmm.tile([P, S], F32, tag="mm")
                    nc.tensor.matmul(
                        ps0,
                        kbf[0:64, c, tcn * P : (tcn + 1) * P],
                        qbf[0:64, c, :],
                        start=True,
                        stop=True,
                    )
                    ps1 = pmm.tile([P, S], F32, tag="mm")
                    nc.tensor.matmul(
                        ps1,
                        kbf[64:128, c, tcn * P : (tcn + 1) * P],
                        qbf[64:128, c, :],
                        start=True,
                        stop=True,
                    )
                    nc.scalar.activation(e0[:, tcn, :], ps0, AF.Exp, scale=0.125)
                    nc.scalar.activation(e1[:, tcn, :], ps1, AF.Exp, scale=0.125)
                pa0 = pmm.tile([65, S], F32, tag="mm", name=f"av{slot}_{h0}")
                pa1 = pmm.tile([65, S], F32, tag="mm", name=f"av{slot}_{h1}")
                for tcn in range(4):
                    nc.tensor.matmul(
                        pa0, vaug[:, tcn, h0, :], e0[:, tcn, :],
                        start=(tcn == 0), stop=(tcn == 3),
                    )
                    nc.tensor.matmul(
                        pa1, vaug[:, tcn, h1, :], e1[:, tcn, :],
                        start=(tcn == 0), stop=(tcn == 3),
                    )
                for pa, par in ((pa0, 0), (pa1, 64)):
                    rec = spool.tile([65, S], BF16, tag="rec")
                    with nc.allow_low_precision(reason="softmax denom recip bf16"):
                        nc.vector.reciprocal(rec[64:65, :], pa[64:65, :])
                    pb = pbc.tile([64, S], F32, tag="bc")
                    nc.tensor.matmul(
                        pb, ones_bf[64:65, 0:64], rec[64:65, :], start=True, stop=True
                    )
                    nmr = spool.tile([64, S], F32, tag="nmr")
                    nc.scalar.activation(nmr, pa[0:64, :], AF.Copy)
                    if par == 0:
                        nc.vector.tensor_mul(attnout2[0:64, c, :], nmr, pb)
                    else:
                        odd = spool.tile([64, S], BF16, tag="odd")
                        nc.vector.tensor_mul(odd, nmr, pb)
                        nc.sync.dma_start(out=attnout2[64:128, c, :], in_=odd)

            # ---- o-proj + residual + rms (fo-major: staggers the rms chain)
            r1 = apool.tile([P, 4, S], BF16, tag="r1")
            for fo in range(4):
                ps = pmm.tile([P, S], F32, tag="mm")
                for c in range(4):
                    nc.tensor.matmul(
                        ps,
                        w_o[:, c, fo * P : (fo + 1) * P],
                        attnout2[:, c, :],
                        start=(c == 0),
                        stop=(c == 3),
                    )
                nc.vector.tensor_add(r1[:, fo, :], ps, hs_bf[:, fo, :])
            pb1 = rms_bcast(r1, 4, 1.0 / HSUB)
            hs2 = apool.tile([P, 4, S], BF16, tag="hs2")
            for fo in range(4):
                nc.vector.tensor_mul(hs2[:, fo, :], r1[:, fo, :], pb1)

            # ---- MLP: silu(gate) * up, then mlp-down fo-major
            gu = apool.tile([P, 12, S], BF16, tag="gu")
            for ic in range(12):
                psg = pmm.tile([P, S], F32, tag="mm")
                for fi in range(4):
                    nc.tensor.matmul(
                        psg,
                        w_gu[:, fi, ic * P : (ic + 1) * P],
                        hs2[:, fi, :],
                        start=(fi == 0),
                        stop=(fi == 3),
                    )
                g32 = spool.tile([P, S], F32, tag="g32")
                nc.scalar.activation(g32, psg, AF.Silu)
                psu = pmm.tile([P, S], F32, tag="mm")
                for fi in range(4):
                    nc.tensor.matmul(
                        psu,
                        w_gu[:, fi, INTER + ic * P : INTER + (ic + 1) * P],
                        hs2[:, fi, :],
                        start=(fi == 0),
                        stop=(fi == 3),
                    )
                nc.vector.tensor_mul(gu[:, ic, :], g32, psu)

            r2 = apool.tile([P, 4, S], BF16, tag="r2")
            for fo in range(4):
                ps = pmm.tile([P, S], F32, tag="mm")
                for ic in range(12):
                    nc.tensor.matmul(
                        ps,
                        w_md[:, ic, fo * P : (fo + 1) * P],
                        gu[:, ic, :],
                        start=(ic == 0),
                        stop=(ic == 11),
                    )
                nc.vector.tensor_add(r2[:, fo, :], ps, hs2[:, fo, :])
            pb2 = rms_bcast(r2, 4, 1.0 / HSUB)
            hs3 = apool.tile([P, 4, S], BF16, tag="hs3")
            for fo in range(4):
                nc.vector.tensor_mul(hs3[:, fo, :], r2[:, fo, :], pb2)

            # ---- up-proj (router weight pre-folded into uwT); accumulate.
            # slot 0 adds the h residual per-chunk (JIT DMA on Pool queue)
            for half in range(2):
                po4 = [
                    pmm.tile([P, S], F32, tag="mm", name=f"ups{slot}_{half}_{i}")
                    for i in range(4)
                ]
                for fi in range(4):
                    for i in range(4):
                        fo = half * 4 + i
                        nc.tensor.matmul(
                            po4[i],
                            w_up[:, fi, fo * P : (fo + 1) * P],
                            hs3[:, fi, :],
                            start=(fi == 0),
                            stop=(fi == 3),
                        )
                for i in range(4):
                    fo = half * 4 + i
                    if slot == 0:
                        h32c = spool.tile([P, S], F32, tag="h32c")
                        nc.gpsimd.dma_start(out=h32c, in_=hT32_e[:, fo, :])
                        nc.vector.tensor_add(acc[:, fo, :], po4[i], h32c)
                    else:
                        nc.vector.tensor_add(acc[:, fo, :], acc[:, fo, :], po4[i])

        # ---- final rms over H, write out (DMAs split over two queues)
        pb3 = rms_bcast(acc, 8, 1.0 / H, final=True)
        # stage the broadcast in SBUF so the 8 muls run all-SBUF (DVE 2x)
        pb3s = otpool.tile([P, S], F32, tag="pb3s", bufs=1)
        nc.scalar.activation(pb3s, pb3, AF.Copy)
        for fo in range(8):
            ot = otpool.tile([P, S], F32, tag="ot")
            nc.vector.tensor_mul(ot, acc[:, fo, :], pb3s)
            eng = nc.sync if fo % 2 == 0 else nc.gpsimd
            eng.dma_start(out=out_e[:, fo, :], in_=ot)

    _orig_to_json = nc.to_json_bytes
    nc.to_json_bytes = lambda: _legalize_bir_json(_orig_to_json())
    return nc


# ------------------------------------------------------------------ host ---
def _fm_tiles(wT, nfi):
    """[Fin, Fout] -> [128, nfi, Fout] feature-major lhsT tiling."""
    fin, fout = wT.shape
    assert fin == nfi * P
    return np.ascontiguousarray(wT.reshape(nfi, P, fout).transpose(1, 0, 2))


def _rot_mat():
    """rotate_half as a [128,128] matrix covering a pair of 64-dim heads.

    rot(q)[d] = -q[d+32] for d%64 < 32 else q[d-32]; lhsT = R.T.
    """
    R = np.zeros((P, P), np.float32)
    for b in (0, 64):
        for d in range(32):
            R[b + d, b + d + 32] = -1.0
            R[b + d + 32, b + d] = 1.0
    return np.ascontiguousarray(R.T).astype(BF)


def _prep_expert(e, down_w, qkv_w, o_w, gate_up_w, mlp_down_w):
    dw = _fm_tiles(down_w[e].T.astype(np.float32), 8).astype(BF)
    qkv = _fm_tiles(qkv_w[e].T.astype(np.float32), 4).astype(BF)
    ow = _fm_tiles(o_w[e].T.astype(np.float32), 4).astype(BF)
    gu = _fm_tiles(gate_up_w[e].T.astype(np.float32), 4).astype(BF)
    md = _fm_tiles(mlp_down_w[e].T.astype(np.float32), 12).astype(BF)
    return dw, qkv, ow, gu, md


def prepare(
    hidden_states,
    input_injection,
    cos,
    sin,
    gate_w,
    down_w,
    up_w,
    qkv_w,
    o_w,
    gate_up_w,
    mlp_down_w,
):
    """Host-side routing + per-core input assembly. Returns (in_maps, aux)."""
    f = np.float32
    hidden_states = np.asarray(hidden_states, f)
    input_injection = np.asarray(input_injection, f)
    cos, sin = np.asarray(cos, f), np.asarray(sin, f)
    gate_w = np.asarray(gate_w, f)
    down_w, up_w = np.asarray(down_w, f), np.asarray(up_w, f)
    qkv_w, o_w = np.asarray(qkv_w, f), np.asarray(o_w, f)
    gate_up_w, mlp_down_w = np.asarray(gate_up_w, f), np.asarray(mlp_down_w, f)

    h = hidden_states + input_injection  # [B, S, H]

    # ---- router on host (tiny): softmax over [B, E], top-2, aux loss
    logits = h[:, 0] @ gate_w.T
    logits = logits - logits.max(axis=-1, keepdims=True)
    ex = np.exp(logits)
    probs = ex / ex.sum(axis=-1, keepdims=True)  # [B, E] f32
    idx = np.argsort(-probs, axis=-1, kind="stable")[:, :TOPK]
    rows = np.arange(B)[:, None]
    topk_vals = probs[rows, idx]
    topk_w = topk_vals / np.clip(topk_vals.sum(-1, keepdims=True), 1e-8, None)
    importance = probs.sum(0) / B
    sel = np.zeros((B, E), f)
    sel[rows, idx] = 1.0
    load = sel.sum(0) / (B * TOPK)
    aux_loss = np.float32((E * importance * load).sum())

    # ---- per-core input assembly
    eprep = {}
    for e in np.unique(idx):
        eprep[int(e)] = _prep_expert(
            int(e), down_w, qkv_w, o_w, gate_up_w, mlp_down_w
        )

    cos2 = np.ascontiguousarray(np.tile(cos.T, (2, 1))).astype(BF)  # [128, 512]
    sin2 = np.ascontiguousarray(np.tile(sin.T, (2, 1))).astype(BF)
    rotT = _rot_mat()

    in_maps = []
    for b in range(B):
        hT = np.ascontiguousarray(h[b].T.reshape(8, P, S).transpose(1, 0, 2))
        dws, qkvs, ows, gus, mds, ups = [], [], [], [], [], []
        for k in range(TOPK):
            e, w = int(idx[b, k]), topk_w[b, k]
            dw, qkv, ow, gu, md = eprep[e]
            dws.append(dw)
            qkvs.append(qkv)
            ows.append(ow)
            gus.append(gu)
            mds.append(md)
            ups.append(_fm_tiles((up_w[e].T * w).astype(np.float32), 4).astype(BF))
        in_maps.append(
            {
                "hTbf": hT.astype(BF),
                "hT32": hT,
                "cos2": cos2,
                "sin2": sin2,
                "rotT": rotT,
                "dwT": np.stack(dws),
                "qkvT": np.stack(qkvs),
                "owT": np.stack(ows),
                "guwT": np.stack(gus),
                "mdwT": np.stack(mds),
                "uwT": np.stack(ups),
            }
        )
    return in_maps, aux_loss


def get_nc(reps=1):
    key = f"bass_nc_{reps}"
    if key not in _CACHE:
        _CACHE[key] = _build_nc(reps)
    return _CACHE[key]


def assemble_out(results):
    out = np.empty((B, S, H), np.float32)
    for b in range(B):
        o = np.asarray(results[b]["out"], np.float32)  # [128, 8, 512]
        out[b] = o.transpose(1, 0, 2).reshape(H, S).T
    return out


def kernel(_trace=False, _tmpdir=None, **inputs):
    in_maps, aux_loss = prepare(**inputs)
    nc = get_nc()
    res = run_bass_kernel_spmd(
        nc, in_maps, core_ids=list(range(NCORES)), trace=_trace, tmpdir=_tmpdir
    )
    global LAST_RESULT
    LAST_RESULT = res
    return assemble_out(res.results), aux_loss


LAST_RESULT = None


# revision 55
# speedup vs baseline: 2.0618x; 2.0618x over previous
"""MoE routing layer (8 experts, top-2, B=8 seqs) on 8 TRN2 NeuronCores.

Strategy: data-parallel over B. Core b processes sequence b with its two
routed experts. Routing (softmax over an 8x8 logit matrix) + aux_loss are
computed on host; the two selected experts' weights are gathered, transposed
to feature-major lhsT layout, cast to bf16, and shipped per-core. The device
kernel keeps activations feature-major ([feature, token]) so every matmul's
contraction dim lands on SBUF partitions; attention uses transposed scores
(k_fm.T @ q_fm), exp on ScalarE, and a ones-row-augmented V matmul that
produces numerator + softmax denominator in one PSUM tile. RoPE's
rotate_half is one extra TensorE matmul with a constant +-1 permutation
matrix. RMS-norm partition reductions and per-token broadcasts are
ones-vector matmuls on the TensorEngine.
"""

import os
import sys

import numpy as np

for _p in ("/opt/trn_rl_repo",):
    if _p not in sys.path and os.path.isdir(_p):
        sys.path.insert(0, _p)

import json

import ml_dtypes
from contextlib import ExitStack

import concourse.bass as bass
import concourse.mybir as mybir
import concourse.tile as tile
from concourse.bass_utils import run_bass_kernel_spmd

BF16 = mybir.dt.bfloat16
F32 = mybir.dt.float32
AF = mybir.ActivationFunctionType
BF = ml_dtypes.bfloat16

B, S, H = 8, 512, 1024
E, TOPK = 8, 2
NH, HD = 8, 64
HSUB = 512
INTER = 1536
EPS = 1e-5
P = 128
NCORES = 8

_CACHE = {}


def _legalize_bir_json(raw: bytes) -> bytes:
    """Hoist inline sync waits onto standalone EventSemaphore instructions.

    The walrus build in this container rejects instructions whose encoding
    has no room for inline sync-wait commands (DVE TensorTensor, Drain, ...)
    with "Too many sync wait commands". Standalone EventSemaphore waits on
    the same engine are always legal, so move every wait there.
    """
    d = json.loads(raw)
    n = 0
    for fn in d["functions"]:
        for blk in fn["blocks"]:
            out = []
            for inst in blk["instructions"]:
                si = inst.get("sync_info")
                waits = si.get("on_wait") if si else None
                if waits and not (
                    inst.get("opcode") == "EventSemaphore" and len(waits) <= 1
                ):
                    for w in waits:
                        n += 1
                        out.append(
                            {
                                "debug": inst.get("debug"),
                                "engine": inst["engine"],
                                "ins": [],
                                "outs": [],
                                "name": f"I-hw{n}",
                                "opcode": "EventSemaphore",
                                "sync_info": {"on_update": [], "on_wait": [w]},
                            }
                        )
                    si["on_wait"] = []
                out.append(inst)
            blk["instructions"] = out
    return json.dumps(d).encode()


# ---------------------------------------------------------------- device ---
def _build_nc(reps=1):
    nc = bass.Bass()

    hTbf_e = nc.declare_dram_parameter("hTbf", [P, 8, S], BF16, isOutput=False)
    hT32_e = nc.declare_dram_parameter("hT32", [P, 8, S], F32, isOutput=False)
    cos2_e = nc.declare_dram_parameter("cos2", [P, S], BF16, isOutput=False)
    sin2_e = nc.declare_dram_parameter("sin2", [P, S], BF16, isOutput=False)
    rotT_e = nc.declare_dram_parameter("rotT", [P, P], BF16, isOutput=False)
    dwT_e = nc.declare_dram_parameter("dwT", [2, P, 8, HSUB], BF16, isOutput=False)
    qkvT_e = nc.declare_dram_parameter("qkvT", [2, P, 4, 1536], BF16, isOutput=False)
    owT_e = nc.declare_dram_parameter("owT", [2, P, 4, HSUB], BF16, isOutput=False)
    guwT_e = nc.declare_dram_parameter(
        "guwT", [2, P, 4, 2 * INTER], BF16, isOutput=False
    )
    mdwT_e = nc.declare_dram_parameter("mdwT", [2, P, 12, HSUB], BF16, isOutput=False)
    uwT_e = nc.declare_dram_parameter("uwT", [2, P, 4, H], BF16, isOutput=False)
    out_e = nc.declare_dram_parameter("out", [P, 8, S], F32, isOutput=True)

    with tile.TileContext(nc) as tc, ExitStack() as ctx:
        cpool = ctx.enter_context(tc.tile_pool(name="c", bufs=1))
        wpool = ctx.enter_context(tc.tile_pool(name="w", bufs=1))
        apool = ctx.enter_context(tc.tile_pool(name="a", bufs=1))
        spool = ctx.enter_context(tc.tile_pool(name="s", bufs=2))
        otpool = ctx.enter_context(tc.tile_pool(name="ot", bufs=4))
        pmm = ctx.enter_context(tc.tile_pool(name="pmm", bufs=6, space="PSUM"))
        pred = ctx.enter_context(tc.tile_pool(name="pred", bufs=1, space="PSUM"))
        pbc = ctx.enter_context(tc.tile_pool(name="pbc", bufs=1, space="PSUM"))

        # ---- persistent tiles. h and small constants ride the Pool-engine
        # DMA queue so expert weights on the sync queue aren't stuck behind
        # them at kernel start.
        t_hbf = cpool.tile([P, 8, S], BF16, tag="hbf")
        t_cos = cpool.tile([P, S], BF16, tag="cos")
        t_sin = cpool.tile([P, S], BF16, tag="sin")
        t_rot = cpool.tile([P, P], BF16, tag="rot")
        ones_bf = cpool.tile([P, P], BF16, tag="ones_bf")
        nc.vector.memset(ones_bf, 1.0)
        ones_f = cpool.tile([P, P], F32, tag="ones_f")
        nc.vector.memset(ones_f, 1.0)
        eps_t = cpool.tile([1, 1], F32, tag="eps")
        nc.vector.memset(eps_t, EPS)
        acc = None

        def rms_bcast(x_tile, nch, scale, final=False):
            """x_tile [P, nch, S]: return PSUM [P, S] f32 = rstd broadcast."""
            red = pred.tile([1, S], F32, tag="red")
            for c in range(nch):
                sq = spool.tile([P, S], BF16, tag="sq")
                if x_tile.dtype == BF16:
                    # all-bf16 SBUF -> DVE 4x mode
                    nc.vector.tensor_mul(sq, x_tile[:, c, :], x_tile[:, c, :])
                else:
                    nc.scalar.activation(sq, x_tile[:, c, :], AF.Square)
                nc.tensor.matmul(
                    red, ones_bf[:, 0:1], sq, start=(c == 0), stop=(c == nch - 1)
                )
            pb = pbc.tile([P, S], F32, tag="bc")
            if final:
                std = spool.tile([1, S], F32, tag="std")
                nc.scalar.activation(
                    std, red, AF.Sqrt, bias=eps_t[0:1, 0:1], scale=float(scale)
                )
                rstd = spool.tile([1, S], F32, tag="std")
                nc.vector.reciprocal(rstd, std)
                nc.tensor.matmul(pb, ones_f[0:1, 0:P], rstd, start=True, stop=True)
            else:
                std = spool.tile([1, S], F32, tag="std")
                nc.scalar.activation(
                    std, red, AF.Sqrt, bias=eps_t[0:1, 0:1], scale=float(scale)
                )
                rstd = spool.tile([1, S], BF16, tag="std")
                with nc.allow_low_precision(reason="intermediate rms rstd bf16"):
                    nc.vector.reciprocal(rstd, std)
                nc.tensor.matmul(pb, ones_bf[0:1, 0:P], rstd, start=True, stop=True)
            return pb

        for rep in range(reps):
          # acc accumulates h + both experts (h added per-chunk at slot 0)
          if acc is None:
              acc = cpool.tile([P, 8, S], F32, tag="acc", name="acc")
          for slot in range(2):
            # ---- expert weights (streamed; tag-slot reuse pipelines the DMA)
            w_dw = wpool.tile([P, 8, HSUB], BF16, tag="dwT")
            if rep == 0 and slot == 0:
                # interleave dw/h chunks so the fi-outer down-proj can start
                # after the first pair and never outruns the DMA feed
                for fi in range(0, 8, 2):
                    nc.sync.dma_start(
                        out=w_dw[:, fi : fi + 2, :], in_=dwT_e[slot][:, fi : fi + 2, :]
                    )
                    nc.sync.dma_start(
                        out=t_hbf[:, fi : fi + 2, :], in_=hTbf_e[:, fi : fi + 2, :]
                    )
            else:
                # later experts: dw/qkv ride the Pool queue, parallel to the
                # sync queue still draining the previous expert's gu/md/up
                nc.gpsimd.dma_start(out=w_dw[:, 0:2, :], in_=dwT_e[slot][:, 0:2, :])
                nc.gpsimd.dma_start(out=w_dw[:, 2:8, :], in_=dwT_e[slot][:, 2:8, :])
            w_qkv = wpool.tile([P, 4, 1536], BF16, tag="qkvT")
            qkv_eng = nc.sync if (rep == 0 and slot == 0) else nc.gpsimd
            qkv_eng.dma_start(out=w_qkv, in_=qkvT_e[slot])
            if rep == 0 and slot == 0:
                # constants are first needed at RoPE, ~15us in — keep them
                # behind the first expert's critical dw/h chunks
                nc.gpsimd.dma_start(out=t_cos, in_=cos2_e[:, :])
                nc.gpsimd.dma_start(out=t_sin, in_=sin2_e[:, :])
                nc.gpsimd.dma_start(out=t_rot, in_=rotT_e[:, :])
            w_o = wpool.tile([P, 4, HSUB], BF16, tag="owT")
            nc.sync.dma_start(out=w_o, in_=owT_e[slot])
            w_gu = wpool.tile([P, 4, 2 * INTER], BF16, tag="guwT")
            nc.sync.dma_start(out=w_gu[:, :, 0:INTER], in_=guwT_e[slot][:, :, 0:INTER])
            nc.sync.dma_start(
                out=w_gu[:, :, INTER : 2 * INTER],
                in_=guwT_e[slot][:, :, INTER : 2 * INTER],
            )
            w_md = wpool.tile([P, 12, HSUB], BF16, tag="mdwT")
            nc.sync.dma_start(out=w_md, in_=mdwT_e[slot])
            w_up = wpool.tile([P, 4, H], BF16, tag="uwT")
            nc.sync.dma_start(out=w_up, in_=uwT_e[slot])

            # ---- down-proj: hs = h @ dw.T   (feature-major [HSUB, S])
            # contraction (fi) outermost over 4 held PSUM banks: the first
            # matmuls need only the first h/dw chunks off the DMA queue
            hs_bf = apool.tile([P, 4, S], BF16, tag="hs_bf")
            po4 = [
                pmm.tile([P, S], F32, tag="mm", name=f"dps{slot}_{i}")
                for i in range(4)
            ]
            for fi in range(8):
                for fo in range(4):
                    nc.tensor.matmul(
                        po4[fo],
                        w_dw[:, fi, fo * P : (fo + 1) * P],
                        t_hbf[:, fi, :],
                        start=(fi == 0),
                        stop=(fi == 7),
                    )
            for fo in range(4):
                nc.scalar.activation(hs_bf[:, fo, :], po4[fo], AF.Copy)

            # ---- qkv + RoPE (rotate_half = matmul with constant +-1 perm)
            qbf = apool.tile([P, 4, S], BF16, tag="qbf")
            kbf = apool.tile([P, 4, S], BF16, tag="kbf")

            def qkv_chunk(col0):
                ps = pmm.tile([P, S], F32, tag="mm")
                for fi in range(4):
                    nc.tensor.matmul(
                        ps,
                        w_qkv[:, fi, col0 : col0 + P],
                        hs_bf[:, fi, :],
                        start=(fi == 0),
                        stop=(fi == 3),
                    )
                return ps

            # all 8 q/k chunk matmuls first (PE never waits on ACT), then the
            # 8 rot matmuls whose ACT-copied inputs are ready by then
            qraw = apool.tile([P, 4, S], BF16, tag="qraw")
            kraw = apool.tile([P, 4, S], BF16, tag="kraw")
            for c in range(4):
                for raw, base in ((qraw, 0), (kraw, 512)):
                    ps_x = qkv_chunk(base + c * P)
                    nc.scalar.activation(raw[:, c, :], ps_x, AF.Copy)
            for c in range(4):
                for raw, dst in ((qraw, qbf), (kraw, kbf)):
                    ps_r = pmm.tile([P, S], F32, tag="mm")
                    nc.tensor.matmul(ps_r, t_rot, raw[:, c, :], start=True, stop=True)
                    # all-bf16 SBUF operands -> DVE 4x mode on t1 and the add
                    t1 = spool.tile([P, S], BF16, tag="t1")
                    nc.vector.tensor_mul(t1, raw[:, c, :], t_cos)
                    t2 = spool.tile([P, S], BF16, tag="t2")
                    nc.vector.tensor_mul(t2, ps_r, t_sin)
                    nc.vector.tensor_add(dst[:, c, :], t1, t2)

            # ---- v (token-major) + ones column for the softmax denominator
            vaug = apool.tile([P, 4, NH, 65], BF16, tag="vaug")
            nc.vector.memset(vaug[:, :, :, 64:65], 1.0)
            for tcn in range(4):
                ps = pmm.tile([P, S], F32, tag="mm")
                for fi in range(4):
                    nc.tensor.matmul(
                        ps,
                        hs_bf[:, fi, tcn * P : (tcn + 1) * P],
                        w_qkv[:, fi, 1024:1536],
                        start=(fi == 0),
                        stop=(fi == 3),
                    )
                nc.scalar.activation(
                    vaug[:, tcn, :, 0:64],
                    ps.rearrange("p (h d) -> p h d", h=NH),
                    AF.Copy,
                )

            # ---- attention, head-pair interleaved: the even/odd heads' 64-
            # contraction score matmuls sit in PE row groups 0/64 and run
            # concurrently in the array. Pairs pack into attnout2 [128, 4, S]
            # so o-proj contracts over the full 128 partitions.
            attnout2 = apool.tile([P, 4, S], BF16, tag="attnout2")
            for c in range(4):
                h0, h1 = 2 * c, 2 * c + 1
                e0 = spool.tile([P, 4, S], BF16, tag="expT")
                e1 = spool.tile([P, 4, S], BF16, tag="expT2")
                for tcn in range(4):
                    ps0 = pmm.tile([P, S], F32, tag="mm")
                    nc.tensor.matmul(
                        ps0,
                        kbf[0:64, c, tcn * P : (tcn + 1) * P],
                        qbf[0:64, c, :],
                        start=True,
                        stop=True,
                    )
                    ps1 = pmm.tile([P, S], F32, tag="mm")
                    nc.tensor.matmul(
                        ps1,
                        kbf[64:128, c, tcn * P : (tcn + 1) * P],
                        qbf[64:128, c, :],
                        start=True,
                        stop=True,
                    )
                    nc.scalar.activation(e0[:, tcn, :], ps0, AF.Exp, scale=0.125)
                    nc.scalar.activation(e1[:, tcn, :], ps1, AF.Exp, scale=0.125)
                pa0 = pmm.tile([65, S], F32, tag="mm", name=f"av{slot}_{h0}")
                pa1 = pmm.tile([65, S], F32, tag="mm", name=f"av{slot}_{h1}")
                for tcn in range(4):
                    nc.tensor.matmul(
                        pa0, vaug[:, tcn, h0, :], e0[:, tcn, :],
                        start=(tcn == 0), stop=(tcn == 3),
                    )
                    nc.tensor.matmul(
                        pa1, vaug[:, tcn, h1, :], e1[:, tcn, :],
                        start=(tcn == 0), stop=(tcn == 3),
                    )
                for pa, par in ((pa0, 0), (pa1, 64)):
                    rec = spool.tile([65, S], BF16, tag="rec")
                    with nc.allow_low_precision(reason="softmax denom recip bf16"):
                        nc.vector.reciprocal(rec[64:65, :], pa[64:65, :])
                    pb = pbc.tile([64, S], F32, tag="bc")
                    nc.tensor.matmul(
                        pb, ones_bf[64:65, 0:64], rec[64:65, :], start=True, stop=True
                    )
                    nmr = spool.tile([64, S], F32, tag="nmr")
                    nc.scalar.activation(nmr, pa[0:64, :], AF.Copy)
                    if par == 0:
                        nc.vector.tensor_mul(attnout2[0:64, c, :], nmr, pb)
                    else:
                        odd = spool.tile([64, S], BF16, tag="odd")
                        nc.vector.tensor_mul(odd, nmr, pb)
                        nc.sync.dma_start(out=attnout2[64:128, c, :], in_=odd)

            # ---- o-proj + residual + rms (fo-major: staggers the rms chain)
            r1 = apool.tile([P, 4, S], BF16, tag="r1")
            for fo in range(4):
                ps = pmm.tile([P, S], F32, tag="mm")
                for c in range(4):
                    nc.tensor.matmul(
                        ps,
                        w_o[:, c, fo * P : (fo + 1) * P],
                        attnout2[:, c, :],
                        start=(c == 0),
                        stop=(c == 3),
                    )
                nc.vector.tensor_add(r1[:, fo, :], ps, hs_bf[:, fo, :])
            pb1 = rms_bcast(r1, 4, 1.0 / HSUB)
            hs2 = apool.tile([P, 4, S], BF16, tag="hs2")
            for fo in range(4):
                nc.vector.tensor_mul(hs2[:, fo, :], r1[:, fo, :], pb1)

            # ---- MLP: silu(gate) * up, then mlp-down fo-major
            gu = apool.tile([P, 12, S], BF16, tag="gu")
            for ic in range(12):
                psg = pmm.tile([P, S], F32, tag="mm")
                for fi in range(4):
                    nc.tensor.matmul(
                        psg,
                        w_gu[:, fi, ic * P : (ic + 1) * P],
                        hs2[:, fi, :],
                        start=(fi == 0),
                        stop=(fi == 3),
                    )
                g32 = spool.tile([P, S], F32, tag="g32")
                nc.scalar.activation(g32, psg, AF.Silu)
                psu = pmm.tile([P, S], F32, tag="mm")
                for fi in range(4):
                    nc.tensor.matmul(
                        psu,
                        w_gu[:, fi, INTER + ic * P : INTER + (ic + 1) * P],
                        hs2[:, fi, :],
                        start=(fi == 0),
                        stop=(fi == 3),
                    )
                nc.vector.tensor_mul(gu[:, ic, :], g32, psu)

            r2 = apool.tile([P, 4, S], BF16, tag="r2")
            for fo in range(4):
                ps = pmm.tile([P, S], F32, tag="mm")
                for ic in range(12):
                    nc.tensor.matmul(
                        ps,
                        w_md[:, ic, fo * P : (fo + 1) * P],
                        gu[:, ic, :],
                        start=(ic == 0),
                        stop=(ic == 11),
                    )
                nc.vector.tensor_add(r2[:, fo, :], ps, hs2[:, fo, :])
            pb2 = rms_bcast(r2, 4, 1.0 / HSUB)
            hs3 = apool.tile([P, 4, S], BF16, tag="hs3")
            for fo in range(4):
                nc.vector.tensor_mul(hs3[:, fo, :], r2[:, fo, :], pb2)

            # ---- up-proj (router weight pre-folded into uwT); accumulate.
            # slot 0 adds the h residual per-chunk (JIT DMA on Pool queue)
            for half in range(2):
                po4 = [
                    pmm.tile([P, S], F32, tag="mm", name=f"ups{slot}_{half}_{i}")
                    for i in range(4)
                ]
                for fi in range(4):
                    for i in range(4):
                        fo = half * 4 + i
                        nc.tensor.matmul(
                            po4[i],
                            w_up[:, fi, fo * P : (fo + 1) * P],
                            hs3[:, fi, :],
                            start=(fi == 0),
                            stop=(fi == 3),
                        )
                for i in range(4):
                    fo = half * 4 + i
                    if slot == 0:
                        h32c = spool.tile([P, S], F32, tag="h32c")
                        nc.gpsimd.dma_start(out=h32c, in_=hT32_e[:, fo, :])
                        nc.vector.tensor_add(acc[:, fo, :], po4[i], h32c)
                    else:
                        nc.vector.tensor_add(acc[:, fo, :], acc[:, fo, :], po4[i])
                        # final-rms reduction emitted here so its matmuls
                        # interleave with the remaining up-proj in PE order
                        if half == 0 and i == 0:
                            red_f = pred.tile([1, S], F32, tag="red")
                        sqf = spool.tile([P, S], BF16, tag="sq")
                        nc.scalar.activation(sqf, acc[:, fo, :], AF.Square)
                        nc.tensor.matmul(
                            red_f,
                            ones_bf[:, 0:1],
                            sqf,
                            start=(fo == 0),
                            stop=(fo == 7),
                        )

        # ---- final rms over H, write out (DMAs split over two queues)
        std_f = spool.tile([1, S], F32, tag="std")
        nc.scalar.activation(
            std_f, red_f, AF.Sqrt, bias=eps_t[0:1, 0:1], scale=float(1.0 / H)
        )
        rstd_f = spool.tile([1, S], F32, tag="std")
        nc.vector.reciprocal(rstd_f, std_f)
        pb3 = pbc.tile([P, S], F32, tag="bc")
        nc.tensor.matmul(pb3, ones_f[0:1, 0:P], rstd_f, start=True, stop=True)
        # stage the broadcast in SBUF so the 8 muls run all-SBUF (DVE 2x)
        pb3s = otpool.tile([P, S], F32, tag="pb3s", bufs=1)
        nc.scalar.activation(pb3s, pb3, AF.Copy)
        for fo in range(8):
            ot = otpool.tile([P, S], F32, tag="ot")
            nc.vector.tensor_mul(ot, acc[:, fo, :], pb3s)
            eng = nc.sync if fo % 2 == 0 else nc.gpsimd
            eng.dma_start(out=out_e[:, fo, :], in_=ot)

    _orig_to_json = nc.to_json_bytes
    nc.to_json_bytes = lambda: _legalize_bir_json(_orig_to_json())
    return nc


# ------------------------------------------------------------------ host ---
def _fm_tiles(wT, nfi):
    """[Fin, Fout] -> [128, nfi, Fout] feature-major lhsT tiling."""
    fin, fout = wT.shape
    assert fin == nfi * P
    return np.ascontiguousarray(wT.reshape(nfi, P, fout).transpose(1, 0, 2))


def _rot_mat():
    """rotate_half as a [128,128] matrix covering a pair of 64-dim heads.

    rot(q)[d] = -q[d+32] for d%64 < 32 else q[d-32]; lhsT = R.T.
    """
    R = np.zeros((P, P), np.float32)
    for b in (0, 64):
        for d in range(32):
            R[b + d, b + d + 32] = -1.0
            R[b + d + 32, b + d] = 1.0
    return np.ascontiguousarray(R.T).astype(BF)


def _prep_expert(e, down_w, qkv_w, o_w, gate_up_w, mlp_down_w):
    dw = _fm_tiles(down_w[e].T.astype(np.float32), 8).astype(BF)
    qkv = _fm_tiles(qkv_w[e].T.astype(np.float32), 4).astype(BF)
    ow = _fm_tiles(o_w[e].T.astype(np.float32), 4).astype(BF)
    gu = _fm_tiles(gate_up_w[e].T.astype(np.float32), 4).astype(BF)
    md = _fm_tiles(mlp_down_w[e].T.astype(np.float32), 12).astype(BF)
    return dw, qkv, ow, gu, md


def prepare(
    hidden_states,
    input_injection,
    cos,
    sin,
    gate_w,
    down_w,
    up_w,
    qkv_w,
    o_w,
    gate_up_w,
    mlp_down_w,
):
    """Host-side routing + per-core input assembly. Returns (in_maps, aux)."""
    f = np.float32
    hidden_states = np.asarray(hidden_states, f)
    input_injection = np.asarray(input_injection, f)
    cos, sin = np.asarray(cos, f), np.asarray(sin, f)
    gate_w = np.asarray(gate_w, f)
    down_w, up_w = np.asarray(down_w, f), np.asarray(up_w, f)
    qkv_w, o_w = np.asarray(qkv_w, f), np.asarray(o_w, f)
    gate_up_w, mlp_down_w = np.asarray(gate_up_w, f), np.asarray(mlp_down_w, f)

    h = hidden_states + input_injection  # [B, S, H]

    # ---- router on host (tiny): softmax over [B, E], top-2, aux loss
    logits = h[:, 0] @ gate_w.T
    logits = logits - logits.max(axis=-1, keepdims=True)
    ex = np.exp(logits)
    probs = ex / ex.sum(axis=-1, keepdims=True)  # [B, E] f32
    idx = np.argsort(-probs, axis=-1, kind="stable")[:, :TOPK]
    rows = np.arange(B)[:, None]
    topk_vals = probs[rows, idx]
    topk_w = topk_vals / np.clip(topk_vals.sum(-1, keepdims=True), 1e-8, None)
    importance = probs.sum(0) / B
    sel = np.zeros((B, E), f)
    sel[rows, idx] = 1.0
    load = sel.sum(0) / (B * TOPK)
    aux_loss = np.float32((E * importance * load).sum())

    # ---- per-core input assembly
    eprep = {}
    for e in np.unique(idx):
        eprep[int(e)] = _prep_expert(
            int(e), down_w, qkv_w, o_w, gate_up_w, mlp_down_w
        )

    cos2 = np.ascontiguousarray(np.tile(cos.T, (2, 1))).astype(BF)  # [128, 512]
    sin2 = np.ascontiguousarray(np.tile(sin.T, (2, 1))).astype(BF)
    rotT = _rot_mat()

    in_maps = []
    for b in range(B):
        hT = np.ascontiguousarray(h[b].T.reshape(8, P, S).transpose(1, 0, 2))
        dws, qkvs, ows, gus, mds, ups = [], [], [], [], [], []
        for k in range(TOPK):
            e, w = int(idx[b, k]), topk_w[b, k]
            dw, qkv, ow, gu, md = eprep[e]
            dws.append(dw)
            qkvs.append(qkv)
            ows.append(ow)
            gus.append(gu)
            mds.append(md)
            ups.append(_fm_tiles((up_w[e].T * w).astype(np.float32), 4).astype(BF))
        in_maps.append(
            {
                "hTbf": hT.astype(BF),
                "hT32": hT,
                "cos2": cos2,
                "sin2": sin2,
                "rotT": rotT,
                "dwT": np.stack(dws),
                "qkvT": np.stack(qkvs),
                "owT": np.stack(ows),
                "guwT": np.stack(gus),
                "mdwT": np.stack(mds),
                "uwT": np.stack(ups),
            }
        )
    return in_maps, aux_loss


def get_nc(reps=1):
    key = f"bass_nc_{reps}"
    if key not in _CACHE:
        _CACHE[key] = _build_nc(reps)
    return _CACHE[key]


def assemble_out(results):
    out = np.empty((B, S, H), np.float32)
    for b in range(B):
        o = np.asarray(results[b]["out"], np.float32)  # [128, 8, 512]
        out[b] = o.transpose(1, 0, 2).reshape(H, S).T
    return out


def kernel(_trace=False, _tmpdir=None, **inputs):
    in_maps, aux_loss = prepare(**inputs)
    nc = get_nc()
    res = run_bass_kernel_spmd(
        nc, in_maps, core_ids=list(range(NCORES)), trace=_trace, tmpdir=_tmpdir
    )
    global LAST_RESULT
    LAST_RESULT = res
    return assemble_out(res.results), aux_loss


LAST_RESULT = None
